# revision 1
# baseline (speedup 1.0000x reference)
"""Trainium2 Bass kernel for CustomRandomEqualize (histogram equalization).

Strategy (per sharding_hint: "replicate LUT math and shard the per-channel
pixel gather"):
  - The 3x256-entry LUT derivation (histogram -> CDF -> LUT) is tiny; it is
    computed once and replicated to all 8 cores as a small input tensor,
    encoded as 255 monotone thresholds per channel:
        lut[v] == sum_y [v >= T_y]   (exact, since the LUT is monotone)
  - The image-scale work (floor-quantize + per-pixel LUT apply + label
    passthrough, ~400MB of traffic) is row-sharded across the 8 NeuronCores.
  - Each core applies the LUT with a fused scalar_tensor_tensor cascade on
    the Vector engine in bf16 (all values are small integers, exact in bf16).

Shapes are hardcoded for image [6, 2048, 4096] f32 (3 RGB + 3 label chans).
"""

import numpy as np

import concourse.bacc as bacc
import concourse.mybir as mybir
from concourse.tile import TileContext
from concourse import bass_utils

NUM_CH = 6
EQ_CH = 3
H = 2048
W = 4096
NCORES = 8
HSH = H // NCORES          # 256 rows per core
P = 128                    # partitions
F = HSH * W // P           # 8192 free elems per partition
NB = 256                   # histogram bins
NT = 255                   # thresholds per channel
BIG = 1.0e6                # "never" threshold sentinel

_CACHED = {}


def _reference_luts(sample_f32):
    """Exact reference LUT math (int64 on host) for the 3 equalize channels.

    Returns luts[3, 256] int64 -- the shifted+clipped LUT, with the
    step==0 identity fallback folded in.
    """
    v = np.floor(sample_f32).astype(np.int64)  # trunc == floor for >=0
    luts = np.zeros((EQ_CH, NB), np.int64)
    for c in range(EQ_CH):
        hist = np.bincount(v[c].ravel(), minlength=NB).astype(np.int64)
        total = int(hist.sum())
        nz = np.nonzero(hist)[0]
        last_nz = int(nz[-1]) if len(nz) else 0
        step = (total - int(hist[last_nz])) // (NB - 1)
        if step == 0:
            luts[c] = np.arange(NB)
            continue
        cum = np.cumsum(hist)
        lut = (cum + step // 2) // step
        lut_shift = np.concatenate([[0], lut[:-1]])
        luts[c] = np.clip(lut_shift, 0, NB - 1)
    return luts


def _thresholds(luts):
    """luts[3, 256] monotone -> T[3, 255] with lut[v] == sum_y [v >= T_y]."""
    T = np.full((EQ_CH, NT), BIG, np.float32)
    for c in range(EQ_CH):
        lut = luts[c]
        for y in range(1, NB):
            idx = np.nonzero(lut >= y)[0]
            if len(idx):
                T[c, y - 1] = float(idx[0])
    return T


def _build_kernel():
    """Build the SPMD Bass program (one NEFF, run on all 8 cores)."""
    nc = bacc.Bacc("TRN2", target_bir_lowering=False, debug=False,
                   num_devices=NCORES)
    x = nc.dram_tensor("x", [NUM_CH, HSH, W], mybir.dt.float32,
                       kind="ExternalInput")
    thr = nc.dram_tensor("thr", [P, EQ_CH * NT], mybir.dt.float32,
                         kind="ExternalInput")
    y = nc.dram_tensor("y", [NUM_CH, HSH, W], mybir.dt.float32,
                       kind="ExternalOutput")

    AOT = mybir.AluOpType
    TWO23 = float(1 << 23)

    with TileContext(nc) as tc:
        with (
            tc.tile_pool(name="io", bufs=1) as io_pool,
            tc.tile_pool(name="wk", bufs=1) as wk_pool,
        ):  # SBUF/partition: io 2x32KB (pass) + wk ~97KB
            # thresholds: [128, 765] f32, same values in every partition row
            tt = wk_pool.tile([P, EQ_CH * NT], mybir.dt.float32, tag="thr")
            nc.sync.dma_start(tt[:], thr[:])
            # ACT Sign biases: 0.5 - T  (sign(v - T + 0.5) = +-1, never 0)
            bt = wk_pool.tile([P, EQ_CH * NT], mybir.dt.float32, tag="bias")
            nc.vector.tensor_scalar(bt[:], tt[:], -1.0, 0.5,
                                    AOT.mult, AOT.add)

            # label channels: straight passthrough through SBUF
            for t in range(EQ_CH, NUM_CH):
                pt = io_pool.tile([P, F], mybir.dt.float32, tag="pass")
                src = x[t].rearrange("(a p) w -> p a w", p=P)
                dst = y[t].rearrange("(a p) w -> p a w", p=P)
                pt3 = pt[:].rearrange("p (a w) -> p a w", w=W)
                nc.sync.dma_start(pt3, src)
                nc.sync.dma_start(dst, pt3)

            for c in range(EQ_CH):
                xf = wk_pool.tile([P, F], mybir.dt.float32, tag="xf")
                src = x[c].rearrange("(a p) w -> p a w", p=P)
                nc.sync.dma_start(xf[:].rearrange("p (a w) -> p a w", w=W), src)

                # floor(x): round-to-nearest via +-2^23, then fix up
                rf = wk_pool.tile([P, F], mybir.dt.float32, tag="rf")
                vb = wk_pool.tile([P, F], mybir.dt.bfloat16, tag="vb")
                nc.vector.tensor_scalar(rf[:], xf[:], TWO23, TWO23,
                                        AOT.add, AOT.subtract)
                nc.vector.tensor_tensor(vb[:], rf[:], xf[:], AOT.is_gt)
                nc.vector.tensor_tensor(rf[:], rf[:], vb[:], AOT.subtract)
                nc.vector.tensor_copy(vb[:], rf[:])

                # threshold cascade, split across engines:
                #   ScalarE: sm_y = sign(v - T_y + 0.5) in {-1, +1}
                #   VectorE: acc += sm_y            (bf16, 2x mode)
                # then lut[v] = (acc + NT) / 2      (exact: small ints in bf16)
                acc = wk_pool.tile([P, F], mybir.dt.bfloat16, tag="acc")
                tmp0 = wk_pool.tile([P, F], mybir.dt.bfloat16, tag="tmp0")
                tmp1 = wk_pool.tile([P, F], mybir.dt.bfloat16, tag="tmp1")
                tmps = [tmp0, tmp1]
                # ACT path contributes sign() in {-1,+1}; DVE path
                # contributes [v >= T] in {0,1}.  With A thresholds on the
                # ACT path:  acc_raw = 2*lut_act - A + lut_dve
                # We rescale DVE terms by 2 (ts2 fused) so everything is in
                # "sign units": acc = 2*lut - A_count  ->  lut = (acc+A)/2.
                act_ys = [yy for yy in range(NT) if yy % 3 != 0]
                dve_ys = [yy for yy in range(NT) if yy % 3 == 0]
                accd = wk_pool.tile([P, F], mybir.dt.bfloat16, tag="accd")
                dtmp = wk_pool.tile([P, F], mybir.dt.bfloat16, tag="dtmp")
                # single interleaved emission: ACT Sign ops (2 bufs) overlap
                # the serial DVE add-chain; DVE-own compare pairs fill the
                # gaps where DVE would otherwise wait on ACT.
                first = True
                firstd = True
                ka = 0
                for yy in range(NT):
                    if yy % 3 == 0:
                        s = tt[:, c * NT + yy: c * NT + yy + 1]
                        if firstd:
                            nc.vector.tensor_scalar(accd[:], vb[:], s, None,
                                                    AOT.is_ge)
                            firstd = False
                        else:
                            nc.vector.tensor_scalar(dtmp[:], vb[:], s, None,
                                                    AOT.is_ge)
                            nc.vector.tensor_tensor(accd[:], accd[:],
                                                    dtmp[:], AOT.add)
                    else:
                        b = bt[:, c * NT + yy: c * NT + yy + 1]
                        tmp = tmps[ka % 2]
                        ka += 1
                        dst = acc if first else tmp
                        nc.scalar.activation(
                            dst[:], vb[:],
                            mybir.ActivationFunctionType.Sign, bias=b)
                        if not first:
                            nc.vector.tensor_tensor(acc[:], acc[:], tmp[:],
                                                    AOT.add)
                        first = False
                # lut = (acc + A)/2 + accd   (all partials bf16-exact)
                nc.vector.tensor_scalar(acc[:], acc[:], float(len(act_ys)),
                                        0.5, AOT.add, AOT.mult)
                nc.vector.tensor_tensor(acc[:], acc[:], accd[:], AOT.add)

                # cast back to f32 on the way out (SWDGE casting DMA)
                dst = y[c].rearrange("(a p) w -> p a w", p=P)
                nc.gpsimd.dma_start(dst, acc[:].rearrange("p (a w) -> p a w", w=W))

    nc.finalize()
    return nc


def kernel(image: np.ndarray) -> np.ndarray:
    image = np.ascontiguousarray(image, dtype=np.float32)
    assert image.shape == (NUM_CH, H, W)

    # ---- replicated LUT math (tiny: 3 x 256) ----
    luts = _reference_luts(image[:EQ_CH])
    T = _thresholds(luts)                                   # [3, 255] f32
    thr_tile = np.ascontiguousarray(np.broadcast_to(
        T.reshape(1, EQ_CH * NT), (P, EQ_CH * NT)).astype(np.float32))

    # ---- build / cache the program ----
    if "nc" not in _CACHED:
        _CACHED["nc"] = _build_kernel()
    nc = _CACHED["nc"]

    # ---- shard rows across the 8 cores ----
    in_maps = []
    for i in range(NCORES):
        shard = np.ascontiguousarray(image[:, i * HSH:(i + 1) * HSH, :])
        in_maps.append({"x": shard, "thr": thr_tile})

    res = bass_utils.run_bass_kernel_spmd(
        nc, in_maps, core_ids=list(range(NCORES)))

    out = np.empty((NUM_CH, H, W), np.float32)
    for i in range(NCORES):
        out[:, i * HSH:(i + 1) * HSH, :] = res.results[i]["y"]
    return out



# revision 2
# speedup vs baseline: 22.7636x; 22.7636x over previous
"""Trainium2 Bass kernel for CustomRandomEqualize (histogram equalization).

Strategy (per sharding_hint: "replicate LUT math and shard the per-channel
pixel gather"):
  - The 3x256-entry LUT derivation (histogram -> CDF -> LUT) is tiny; it is
    computed once on host (replicated, exact int64 math).
  - Key observation: the equalize LUT is monotone with lut[0] == 0, so
        lut[v] = v + sum_t w_t * [v >= tau_t]
    where the (tau_t, w_t) are the jump points of d[v] = lut[v] - v and
    sum_t |w_t| = total variation of d.  For near-uniform histograms d is
    tiny (TV of ~2-4 per channel), so the per-pixel apply collapses to a
    handful of fused compare-accumulate ops instead of a 255-entry cascade.
  - Since tau is an integer and x >= 0:  [floor(x) >= tau] == [x >= tau],
    so the cascade runs directly on the raw f32 pixels; floor(x) itself is
    round-to-nearest via +-2^23 plus a fused [x >= r] correction term.
  - The image-scale work is row-sharded across the 8 NeuronCores; the 3
    label channels pass through on a separate DMA queue.

The slot structure is data-dependent, so the program is built (and cached)
per distinct slot structure; thresholds are baked as immediates.

Shapes are hardcoded for image [6, 2048, 4096] f32 (3 RGB + 3 label chans).
"""

import numpy as np

import concourse.bacc as bacc
import concourse.mybir as mybir
from concourse.tile import TileContext
from concourse import bass_utils

NUM_CH = 6
EQ_CH = 3
H = 2048
W = 4096
NCORES = 8
HSH = H // NCORES          # 256 rows per core
P = 128                    # partitions
NB = 256                   # histogram bins
CW = 2048                  # free-dim chunk (8KB/partition f32)
TWO23 = float(1 << 23)

_CACHED = {}


def _reference_luts(sample_f32):
    """Exact reference LUT math (int64 on host) for the 3 equalize channels.

    Returns luts[3, 256] int64 -- the shifted+clipped LUT, with the
    step==0 identity fallback folded in.
    """
    v = np.floor(sample_f32).astype(np.int64)  # trunc == floor for >=0
    luts = np.zeros((EQ_CH, NB), np.int64)
    for c in range(EQ_CH):
        hist = np.bincount(v[c].ravel(), minlength=NB).astype(np.int64)
        total = int(hist.sum())
        nz = np.nonzero(hist)[0]
        last_nz = int(nz[-1]) if len(nz) else 0
        step = (total - int(hist[last_nz])) // (NB - 1)
        if step == 0:
            luts[c] = np.arange(NB)
            continue
        cum = np.cumsum(hist)
        lut = (cum + step // 2) // step
        lut_shift = np.concatenate([[0], lut[:-1]])
        luts[c] = np.clip(lut_shift, 0, NB - 1)
    return luts


def _slots(luts):
    """Per-channel unit step terms of d[v] = lut[v] - v.

    Returns (slots, consts): slots[c] is a tuple of (tau, is_ge) unit
    steps with lut[v] = v + sum_t (+1 if is_ge: [v >= tau]) + (+1 if
    not is_ge: [v < tau]) + consts[c] + ([x >= rne(x)] - 1) folded in.
    """
    slots = []
    consts = []
    for c in range(EQ_CH):
        d = luts[c] - np.arange(NB)
        assert d[0] == 0
        jumps = np.diff(d)
        sl = []
        neg = 0
        for tau in np.nonzero(jumps)[0]:
            w = int(jumps[tau])
            for _ in range(abs(w)):
                if w > 0:
                    sl.append((int(tau) + 1, True))    # +[v >= tau+1]
                else:
                    sl.append((int(tau) + 1, False))   # +[v < tau+1], -1
                    neg += 1
        slots.append(tuple(sl))
        consts.append(float(-1 - neg))  # -1 folds the rne fixup [x>=r]-1
    return tuple(slots), tuple(consts)


def _build_kernel(slots, consts):
    """Build the SPMD Bass program (one NEFF, run on all 8 cores)."""
    nc = bacc.Bacc("TRN2", target_bir_lowering=False, debug=False,
                   num_devices=NCORES)
    x = nc.dram_tensor("x", [NUM_CH, HSH, W], mybir.dt.float32,
                       kind="ExternalInput")
    y = nc.dram_tensor("y", [NUM_CH, HSH, W], mybir.dt.float32,
                       kind="ExternalOutput")

    AOT = mybir.AluOpType
    f32 = mybir.dt.float32

    with TileContext(nc) as tc:
        with (
            tc.tile_pool(name="io", bufs=3) as io_pool,
            tc.tile_pool(name="wk", bufs=2) as wk_pool,
            tc.tile_pool(name="lb", bufs=2) as lb_pool,
        ):
            # label channels: straight passthrough via SBUF on the gpsimd
            # DMA queue, overlapped with the equalize pipeline below
            for t in range(EQ_CH, NUM_CH):
                for h0 in range(0, HSH, P):
                    lt = lb_pool.tile([P, W], f32, tag="l")
                    nc.gpsimd.dma_start(lt[:], x[t, h0:h0 + P, :])
                    nc.gpsimd.dma_start(y[t, h0:h0 + P, :], lt[:])

            for c in range(EQ_CH):
                for h0 in range(0, HSH, P):
                    for w0 in range(0, W, CW):
                        xt = io_pool.tile([P, CW], f32, tag="x")
                        nc.sync.dma_start(xt[:], x[c, h0:h0 + P, w0:w0 + CW])
                        # r = round-to-nearest(x) via +-2^23
                        rt = wk_pool.tile([P, CW], f32, tag="r")
                        nc.vector.tensor_scalar(rt[:], xt[:], TWO23, TWO23,
                                                AOT.add, AOT.subtract)
                        # a = [x >= r]  (1 - round-up fixup)
                        at = wk_pool.tile([P, CW], f32, tag="a")
                        nc.vector.tensor_tensor(at[:], xt[:], rt[:], AOT.is_ge)
                        # cascade: a += [x >= tau] or [x < tau]
                        for tau, ge in slots[c]:
                            nc.vector.scalar_tensor_tensor(
                                at[:], xt[:], float(tau), at[:],
                                AOT.is_ge if ge else AOT.is_lt, AOT.add)
                        # out = (r + C) + a
                        ot = io_pool.tile([P, CW], f32, tag="o")
                        nc.vector.scalar_tensor_tensor(
                            ot[:], rt[:], consts[c], at[:], AOT.add, AOT.add)
                        nc.scalar.dma_start(y[c, h0:h0 + P, w0:w0 + CW], ot[:])

    nc.finalize()
    return nc


def _prepare(image):
    """Host-side LUT math + program build (cached by slot structure)."""
    luts = _reference_luts(image[:EQ_CH])
    slots, consts = _slots(luts)
    key = (slots, consts)
    if key not in _CACHED:
        _CACHED[key] = _build_kernel(slots, consts)
    return _CACHED[key]


def _in_maps(image):
    return [{"x": np.ascontiguousarray(image[:, i * HSH:(i + 1) * HSH, :])}
            for i in range(NCORES)]


def kernel(image: np.ndarray) -> np.ndarray:
    image = np.ascontiguousarray(image, dtype=np.float32)
    assert image.shape == (NUM_CH, H, W)

    nc = _prepare(image)
    res = bass_utils.run_bass_kernel_spmd(
        nc, _in_maps(image), core_ids=list(range(NCORES)))

    out = np.empty((NUM_CH, H, W), np.float32)
    for i in range(NCORES):
        out[:, i * HSH:(i + 1) * HSH, :] = res.results[i]["y"]
    return out


# revision 8
# speedup vs baseline: 24.6454x; 1.0827x over previous
"""Trainium2 Bass kernel for CustomRandomEqualize (histogram equalization).

Strategy (per sharding_hint: "replicate LUT math and shard the per-channel
pixel gather"):
  - The 3x256-entry LUT derivation (histogram -> CDF -> LUT) is tiny; it is
    computed once on host (replicated, exact int64 math).
  - Key observation: the equalize LUT is monotone with lut[0] == 0, so
        lut[v] = v + sum_t w_t * [v >= tau_t]
    where the (tau_t, w_t) are the jump points of d[v] = lut[v] - v and
    sum_t |w_t| = total variation of d.  For near-uniform histograms d is
    tiny (TV of ~2-4 per channel), so the per-pixel apply collapses to a
    handful of ops instead of a 255-entry threshold cascade.
  - Work is split across engines per 2048-px chunk:
      ACT:  r' = Copy(x + 2^23)               (round-to-nearest, biased)
            s_t = Sign(+-v + (0.5 - tau))     (+-1 indicators on the exact
                                               integer v; is_lt slots fold
                                               their negation into scale)
      DVE:  a0 = [(r' - 2^23) <= x]           (floor fixup, fused stt)
            v  = (r' - 2^23 - 1) + a0         (exact floor, bf16)
            U  = sum_t s_t                    (bf16 adds, 2x mode)
            w  = 0.5*U + cw ; out = v + w     (4x-mode ts + 2x tt)
    with lut[v] = v + (U + K)/2 - #lt folded into the constant cw.
  - The eq result is exact small integers, stored as uint8 (4x less store
    traffic); host upcasts. The 3 label channels pass through via SBUF on
    the gpsimd (SWDGE) queue, overlapped with the equalize pipeline.
  - Image rows are sharded across the 8 NeuronCores.

The slot structure is data-dependent, so the program is built (and cached)
per distinct slot structure; thresholds are baked as immediates.

Shapes are hardcoded for image [6, 2048, 4096] f32 (3 RGB + 3 label chans).
"""

import numpy as np

import concourse.bacc as bacc
import concourse.mybir as mybir
from concourse.tile import TileContext
from concourse import bass_utils

NUM_CH = 6
EQ_CH = 3
H = 2048
W = 4096
NCORES = 8
HSH = H // NCORES          # 256 rows per core
P = 128                    # partitions
NB = 256                   # histogram bins
CW = 2048                  # free-dim chunk (8KB/partition f32)
TWO23 = float(1 << 23)

_CACHED = {}


def _reference_luts(sample_f32):
    """Exact reference LUT math (int64 on host) for the 3 equalize channels.

    Returns luts[3, 256] int64 -- the shifted+clipped LUT, with the
    step==0 identity fallback folded in.
    """
    v = np.floor(sample_f32).astype(np.int64)  # trunc == floor for >=0
    luts = np.zeros((EQ_CH, NB), np.int64)
    for c in range(EQ_CH):
        hist = np.bincount(v[c].ravel(), minlength=NB).astype(np.int64)
        total = int(hist.sum())
        nz = np.nonzero(hist)[0]
        last_nz = int(nz[-1]) if len(nz) else 0
        step = (total - int(hist[last_nz])) // (NB - 1)
        if step == 0:
            luts[c] = np.arange(NB)
            continue
        cum = np.cumsum(hist)
        lut = (cum + step // 2) // step
        lut_shift = np.concatenate([[0], lut[:-1]])
        luts[c] = np.clip(lut_shift, 0, NB - 1)
    return luts


def _slots(luts):
    """Per-channel unit step terms of d[v] = lut[v] - v.

    Returns (slots, consts): slots[c] is a tuple of (tau, is_ge) unit
    steps; consts[c] = K/2 - #lt, so that with U = sum of +-1 signs
    (is_lt slots negated):  lut[v] = v + 0.5*U + consts[c].
    """
    slots = []
    consts = []
    for c in range(EQ_CH):
        d = luts[c] - np.arange(NB)
        assert d[0] == 0
        jumps = np.diff(d)
        sl = []
        for tau in np.nonzero(jumps)[0]:
            w = int(jumps[tau])
            # +[v >= tau+1] per unit, or +[v < tau+1] (and -1 in const)
            sl.extend([(int(tau) + 1, w > 0)] * abs(w))
        nlt = sum(1 for _, ge in sl if not ge)
        slots.append(tuple(sl))
        consts.append(float(len(sl) / 2.0 - nlt))
    return tuple(slots), tuple(consts)


def _build_kernel(slots, consts):
    """Build the SPMD Bass program (one NEFF, run on all 8 cores)."""
    nc = bacc.Bacc("TRN2", target_bir_lowering=False, debug=False,
                   num_devices=NCORES)
    x = nc.dram_tensor("x", [NUM_CH, HSH, W], mybir.dt.float32,
                       kind="ExternalInput")
    ye = nc.dram_tensor("ye", [EQ_CH, HSH, W], mybir.dt.uint8,
                        kind="ExternalOutput")
    yl = nc.dram_tensor("yl", [NUM_CH - EQ_CH, HSH, W], mybir.dt.float32,
                        kind="ExternalOutput")

    AOT = mybir.AluOpType
    ACF = mybir.ActivationFunctionType
    f32 = mybir.dt.float32
    bf16 = mybir.dt.bfloat16

    with TileContext(nc) as tc:
        with (
            tc.tile_pool(name="io", bufs=3) as io_pool,
            tc.tile_pool(name="wk", bufs=2) as wk_pool,
            tc.tile_pool(name="sg", bufs=4) as sg_pool,
            tc.tile_pool(name="lb", bufs=2) as lb_pool,
            tc.tile_pool(name="cst", bufs=1) as cst_pool,
        ):
            # Sign biases must be [P,1] SBUF operands: memset one per slot
            nslots = max(1, sum(len(s) for s in slots))
            bias_t = cst_pool.tile([P, nslots], f32, tag="bias")
            j = 0
            for c in range(EQ_CH):
                for tau, ge in slots[c]:
                    b = (0.5 - tau) if ge else (tau - 0.5)
                    nc.vector.memset(bias_t[:, j:j + 1], float(b))
                    j += 1

            # label channels: straight passthrough via SBUF on the gpsimd
            # (SWDGE) queue, overlapped with the equalize pipeline below
            for t in range(EQ_CH, NUM_CH):
                for h0 in range(0, HSH, P):
                    lt = lb_pool.tile([P, W], f32, tag="l")
                    nc.gpsimd.dma_start(lt[:], x[t, h0:h0 + P, :])
                    nc.gpsimd.dma_start(yl[t - EQ_CH, h0:h0 + P, :], lt[:])

            for c in range(EQ_CH):
                K = len(slots[c])
                base = sum(len(slots[cc]) for cc in range(c))
                for h0 in range(0, HSH, P):
                    for w0 in range(0, W, CW):
                        xt = io_pool.tile([P, CW], f32, tag="x")
                        nc.sync.dma_start(xt[:], x[c, h0:h0 + P, w0:w0 + CW])
                        # r' = round-to-nearest(x) + 2^23  (ACT, exact)
                        rt = wk_pool.tile([P, CW], f32, tag="r")
                        nc.scalar.activation(rt[:], xt[:], ACF.Copy,
                                             bias=TWO23)
                        # a0 = [(r' - 2^23) <= x]  (1 - round-up fixup)
                        at = wk_pool.tile([P, CW], bf16, tag="a")
                        nc.vector.scalar_tensor_tensor(
                            at[:], rt[:], -TWO23, xt[:], AOT.add, AOT.is_le)
                        ot = io_pool.tile([P, CW], mybir.dt.uint8, tag="o")
                        if K == 0:
                            # identity channel: out = v = (r'-2^23-1) + a0
                            nc.vector.scalar_tensor_tensor(
                                ot[:], rt[:], -(TWO23 + 1.0), at[:],
                                AOT.add, AOT.add)
                        else:
                            # v = (r' - 2^23 - 1) + a0   (exact floor, bf16)
                            vt = wk_pool.tile([P, CW], bf16, tag="v")
                            nc.vector.scalar_tensor_tensor(
                                vt[:], rt[:], -(TWO23 + 1.0), at[:],
                                AOT.add, AOT.add)
                            # +-1 indicators on ACT (exact on integer v);
                            # bf16 tree-sum on DVE (2x mode)
                            ut = None
                            st_prev = None
                            for k, (tau, ge) in enumerate(slots[c]):
                                st = sg_pool.tile([P, CW], bf16, tag="s")
                                nc.scalar.activation(
                                    st[:], vt[:], ACF.Sign,
                                    bias=bias_t[:, base + k:base + k + 1],
                                    scale=1.0 if ge else -1.0)
                                if st_prev is None:
                                    st_prev = st
                                elif ut is None:
                                    ut = wk_pool.tile([P, CW], bf16, tag="u")
                                    nc.vector.tensor_tensor(
                                        ut[:], st_prev[:], st[:], AOT.add)
                                else:
                                    nc.vector.tensor_tensor(
                                        ut[:], ut[:], st[:], AOT.add)
                            # w = 0.5*U + cw ; out = v + w
                            usrc = ut if ut is not None else st_prev
                            wt = wk_pool.tile([P, CW], bf16, tag="w")
                            nc.vector.tensor_scalar(
                                wt[:], usrc[:], 0.5, consts[c],
                                AOT.mult, AOT.add)
                            nc.vector.tensor_tensor(
                                ot[:], vt[:], wt[:], AOT.add)
                        nc.sync.dma_start(ye[c, h0:h0 + P, w0:w0 + CW],
                                          ot[:])

    nc.finalize()
    return nc


def _prepare(image):
    """Host-side LUT math + program build (cached by slot structure)."""
    luts = _reference_luts(image[:EQ_CH])
    slots, consts = _slots(luts)
    key = (slots, consts)
    if key not in _CACHED:
        _CACHED[key] = _build_kernel(slots, consts)
    return _CACHED[key]


def _in_maps(image):
    return [{"x": np.ascontiguousarray(image[:, i * HSH:(i + 1) * HSH, :])}
            for i in range(NCORES)]


def kernel(image: np.ndarray) -> np.ndarray:
    image = np.ascontiguousarray(image, dtype=np.float32)
    assert image.shape == (NUM_CH, H, W)

    nc = _prepare(image)
    res = bass_utils.run_bass_kernel_spmd(
        nc, _in_maps(image), core_ids=list(range(NCORES)))

    out = np.empty((NUM_CH, H, W), np.float32)
    for i in range(NCORES):
        sl = slice(i * HSH, (i + 1) * HSH)
        out[:EQ_CH, sl] = res.results[i]["ye"].astype(np.float32)
        out[EQ_CH:, sl] = res.results[i]["yl"]
    return out


# revision 9
# speedup vs baseline: 27.3027x; 1.1078x over previous
"""Trainium2 Bass kernel for CustomRandomEqualize (histogram equalization).

Strategy (per sharding_hint: "replicate LUT math and shard the per-channel
pixel gather"):
  - The 3x256-entry LUT derivation (histogram -> CDF -> LUT) is tiny; it is
    computed once on host (replicated, exact int64 math).
  - Key observation: the equalize LUT is monotone with lut[0] == 0, so
        lut[v] = v + sum_t w_t * [v >= tau_t]
    where the (tau_t, w_t) are the jump points of d[v] = lut[v] - v and
    sum_t |w_t| = total variation of d.  For near-uniform histograms d is
    tiny (TV of ~2-4 per channel), so the per-pixel apply collapses to a
    handful of ops instead of a 255-entry threshold cascade.
  - Work is split across engines per 2048-px chunk:
      ACT:  r' = Copy(x + 2^23)               (round-to-nearest, biased;
                                               software-pipelined one chunk
                                               ahead of the DVE stage)
      DVE:  a0 = [(r' - 2^23) <= x]           (floor fixup, fused stt)
            v' = (r' - 2^23 - 1 + cw) + a0    (exact floor + const, f32)
      ACT:  s_t = Sign(+-v' + bias_t)         (+-1 indicators, exact on the
                                               integer-valued v'; is_lt
                                               slots negate via scale=-1)
      DVE:  U = sum_t s_t (bf16 2x adds);  out = 0.5*U + v'  (fused stt)
    with lut[v] = v + (U + K)/2 - #lt via cw = K/2 - #lt - #pad folded
    into v'.  Slot lists are padded to even K so cw is integral.
  - The eq result is exact small integers, stored as uint8 (4x less store
    traffic); host upcasts.  The 3 label channels pass through via SBUF.
  - DMA rings: eq loads on the sync HWDGE ring; eq stores + label
    passthrough on the gpsimd SWDGE ring (so stores never head-block
    loads); ACT's ring carries no DMA.
  - Channels are interleaved chunk-by-chunk to even out the ACT load.
  - Image rows are sharded across the 8 NeuronCores.

The slot structure is data-dependent, so the program is built (and cached)
per distinct slot structure; thresholds are baked as immediates.

Shapes are hardcoded for image [6, 2048, 4096] f32 (3 RGB + 3 label chans).
"""

import numpy as np

import concourse.bacc as bacc
import concourse.mybir as mybir
from concourse.tile import TileContext
from concourse import bass_utils

NUM_CH = 6
EQ_CH = 3
H = 2048
W = 4096
NCORES = 8
HSH = H // NCORES          # 256 rows per core
P = 128                    # partitions
NB = 256                   # histogram bins
CW = 2048                  # free-dim chunk (8KB/partition f32)
TWO23 = float(1 << 23)

_CACHED = {}


def _reference_luts(sample_f32):
    """Exact reference LUT math (int64 on host) for the 3 equalize channels.

    Returns luts[3, 256] int64 -- the shifted+clipped LUT, with the
    step==0 identity fallback folded in.
    """
    v = np.floor(sample_f32).astype(np.int64)  # trunc == floor for >=0
    luts = np.zeros((EQ_CH, NB), np.int64)
    for c in range(EQ_CH):
        hist = np.bincount(v[c].ravel(), minlength=NB).astype(np.int64)
        total = int(hist.sum())
        nz = np.nonzero(hist)[0]
        last_nz = int(nz[-1]) if len(nz) else 0
        step = (total - int(hist[last_nz])) // (NB - 1)
        if step == 0:
            luts[c] = np.arange(NB)
            continue
        cum = np.cumsum(hist)
        lut = (cum + step // 2) // step
        lut_shift = np.concatenate([[0], lut[:-1]])
        luts[c] = np.clip(lut_shift, 0, NB - 1)
    return luts


def _slots(luts):
    """Per-channel unit step terms of d[v] = lut[v] - v.

    Returns (slots, consts): slots[c] is an even-length tuple of
    (tau, is_ge) unit steps ((0, True) padding always fires);
    consts[c] = cw = K/2 - #lt - #pad, integral, so that
        lut[v] = (v + cw) + 0.5 * sum_t s_t.
    """
    slots = []
    consts = []
    for c in range(EQ_CH):
        d = luts[c] - np.arange(NB)
        assert d[0] == 0
        jumps = np.diff(d)
        sl = []
        for tau in np.nonzero(jumps)[0]:
            w = int(jumps[tau])
            # +[v >= tau+1] per unit, or +[v < tau+1] (and -1 in const)
            sl.extend([(int(tau) + 1, w > 0)] * abs(w))
        npad = 0
        if len(sl) % 2 == 1:
            sl.append((0, True))
            npad = 1
        nlt = sum(1 for _, ge in sl if not ge)
        slots.append(tuple(sl))
        consts.append(float(len(sl) // 2 - nlt - npad))
    return tuple(slots), tuple(consts)


def _build_kernel(slots, consts):
    """Build the SPMD Bass program (one NEFF, run on all 8 cores)."""
    nc = bacc.Bacc("TRN2", target_bir_lowering=False, debug=False,
                   num_devices=NCORES)
    x = nc.dram_tensor("x", [NUM_CH, HSH, W], mybir.dt.float32,
                       kind="ExternalInput")
    ye = nc.dram_tensor("ye", [EQ_CH, HSH, W], mybir.dt.uint8,
                        kind="ExternalOutput")
    yl = nc.dram_tensor("yl", [NUM_CH - EQ_CH, HSH, W], mybir.dt.float32,
                        kind="ExternalOutput")

    AOT = mybir.AluOpType
    ACF = mybir.ActivationFunctionType
    f32 = mybir.dt.float32
    bf16 = mybir.dt.bfloat16

    # interleaved chunk schedule: channel innermost
    chunks = [(c, h0, w0)
              for h0 in range(0, HSH, P)
              for w0 in range(0, W, CW)
              for c in range(EQ_CH)]
    # label passthrough transfers, interleaved ~1 per 2 chunks
    labels = [(t, h0) for t in range(EQ_CH, NUM_CH)
              for h0 in range(0, HSH, P)]

    with TileContext(nc) as tc:
        with (
            tc.tile_pool(name="io", bufs=4) as io_pool,
            tc.tile_pool(name="wk", bufs=3) as wk_pool,
            tc.tile_pool(name="sg", bufs=4) as sg_pool,
            tc.tile_pool(name="lb", bufs=2) as lb_pool,
            tc.tile_pool(name="cst", bufs=1) as cst_pool,
        ):
            # Sign biases must be [P,1] SBUF operands: memset one per slot
            nslots = max(1, sum(len(s) for s in slots))
            bias_t = cst_pool.tile([P, nslots], f32, tag="bias")
            j = 0
            for c in range(EQ_CH):
                for tau, ge in slots[c]:
                    b = (0.5 - tau - consts[c]) if ge \
                        else (tau - 0.5 + consts[c])
                    nc.vector.memset(bias_t[:, j:j + 1], float(b))
                    j += 1

            def emit_load(i):
                """Stage A: DMA load + ACT r' (pipelined one chunk ahead)."""
                c, h0, w0 = chunks[i]
                xt = io_pool.tile([P, CW], f32, tag="x", name=f"x{i}")
                nc.sync.dma_start(xt[:], x[c, h0:h0 + P, w0:w0 + CW])
                rt = wk_pool.tile([P, CW], f32, tag="r", name=f"r{i}")
                nc.scalar.activation(rt[:], xt[:], ACF.Copy, bias=TWO23)
                return xt, rt

            def emit_label(t, h0):
                lt = lb_pool.tile([P, W], f32, tag="l")
                nc.gpsimd.dma_start(lt[:], x[t, h0:h0 + P, :])
                nc.gpsimd.dma_start(yl[t - EQ_CH, h0:h0 + P, :], lt[:])

            staged = emit_load(0)
            li = 0
            for i, (c, h0, w0) in enumerate(chunks):
                xt, rt = staged
                if i + 1 < len(chunks):
                    staged = emit_load(i + 1)
                if i % 2 == 0 and li < len(labels):
                    emit_label(*labels[li])
                    li += 1

                K = len(slots[c])
                base = sum(len(slots[cc]) for cc in range(c))
                # a0 = [(r' - 2^23) <= x]  (1 - round-up fixup)
                at = wk_pool.tile([P, CW], bf16, tag="a")
                nc.vector.scalar_tensor_tensor(
                    at[:], rt[:], -TWO23, xt[:], AOT.add, AOT.is_le)
                ot = io_pool.tile([P, CW], mybir.dt.uint8, tag="o")
                if K == 0:
                    # identity channel: out = v = (r'-2^23-1) + a0
                    nc.vector.scalar_tensor_tensor(
                        ot[:], rt[:], -(TWO23 + 1.0), at[:],
                        AOT.add, AOT.add)
                else:
                    # v' = (r' - 2^23 - 1 + cw) + a0  (exact, f32)
                    vt = wk_pool.tile([P, CW], f32, tag="v")
                    nc.vector.scalar_tensor_tensor(
                        vt[:], rt[:], consts[c] - (TWO23 + 1.0), at[:],
                        AOT.add, AOT.add)
                    # +-1 indicators on ACT (exact on integer-valued v');
                    # bf16 tree-sum on DVE (2x mode)
                    ut = None
                    st_prev = None
                    for k, (tau, ge) in enumerate(slots[c]):
                        st = sg_pool.tile([P, CW], bf16, tag="s")
                        nc.scalar.activation(
                            st[:], vt[:], ACF.Sign,
                            bias=bias_t[:, base + k:base + k + 1],
                            scale=1.0 if ge else -1.0)
                        if st_prev is None:
                            st_prev = st
                        elif ut is None:
                            ut = wk_pool.tile([P, CW], bf16, tag="u")
                            nc.vector.tensor_tensor(
                                ut[:], st_prev[:], st[:], AOT.add)
                        else:
                            nc.vector.tensor_tensor(
                                ut[:], ut[:], st[:], AOT.add)
                    # out = 0.5*U + v'
                    usrc = ut if ut is not None else st_prev
                    nc.vector.scalar_tensor_tensor(
                        ot[:], usrc[:], 0.5, vt[:], AOT.mult, AOT.add)
                nc.gpsimd.dma_start(ye[c, h0:h0 + P, w0:w0 + CW], ot[:])

    nc.finalize()
    return nc


def _prepare(image):
    """Host-side LUT math + program build (cached by slot structure)."""
    luts = _reference_luts(image[:EQ_CH])
    slots, consts = _slots(luts)
    key = (slots, consts)
    if key not in _CACHED:
        _CACHED[key] = _build_kernel(slots, consts)
    return _CACHED[key]


def _in_maps(image):
    return [{"x": np.ascontiguousarray(image[:, i * HSH:(i + 1) * HSH, :])}
            for i in range(NCORES)]


def kernel(image: np.ndarray) -> np.ndarray:
    image = np.ascontiguousarray(image, dtype=np.float32)
    assert image.shape == (NUM_CH, H, W)

    nc = _prepare(image)
    res = bass_utils.run_bass_kernel_spmd(
        nc, _in_maps(image), core_ids=list(range(NCORES)))

    out = np.empty((NUM_CH, H, W), np.float32)
    for i in range(NCORES):
        sl = slice(i * HSH, (i + 1) * HSH)
        out[:EQ_CH, sl] = res.results[i]["ye"].astype(np.float32)
        out[EQ_CH:, sl] = res.results[i]["yl"]
    return out


# revision 12
# speedup vs baseline: 30.0523x; 1.1007x over previous
"""Trainium2 Bass kernel for CustomRandomEqualize (histogram equalization).

Strategy (per sharding_hint: "replicate LUT math and shard the per-channel
pixel gather"):
  - The 3x256-entry LUT derivation (histogram -> CDF -> LUT) is tiny; it is
    computed once on host (replicated, exact int64 math).
  - Key observation: the equalize LUT is monotone with lut[0] == 0, so
        lut[v] = v + sum_t w_t * [v >= tau_t]
    where the (tau_t, w_t) are the jump points of d[v] = lut[v] - v and
    sum_t |w_t| = total variation of d.  For near-uniform histograms d is
    tiny (TV of ~2-4 per channel), so the per-pixel apply collapses to a
    handful of ops instead of a 255-entry threshold cascade.
  - Work is split across engines per 2048-px chunk:
      ACT:  r' = Copy(x + 2^23)               (round-to-nearest, biased;
                                               software-pipelined one chunk
                                               ahead of the DVE stage)
      DVE:  a0 = [(r' - 2^23) <= x]           (floor fixup, fused stt)
            v' = (r' - 2^23 - 1 + cw) + a0    (exact floor + const, f32)
      ACT:  s_t = Sign(+-v' + bias_t)         (+-1 indicators, exact on the
                                               integer-valued v'; is_lt
                                               slots negate via scale=-1)
      PE:   U = sum_t s_t                     (identity-weight matmuls
                                               accumulating in PSUM -- the
                                               otherwise-idle TensorE does
                                               the tree sum)
      DVE:  out = 0.5*U + v'                  (single fused stt from PSUM)
    with lut[v] = v + (U + K)/2 - #lt via cw = K/2 - #lt - #pad folded
    into v'.  Slot lists are padded to even K so cw is integral.
  - The eq result is exact small integers, stored as uint8 (4x less store
    traffic); host upcasts.  The 3 label channels pass through via SBUF.
  - DMA rings: eq loads on the sync HWDGE ring; eq stores + label
    passthrough on the gpsimd SWDGE ring (so stores never head-block
    loads); ACT's ring carries no DMA.
  - Channels are interleaved chunk-by-chunk to even out the ACT load.
  - Image rows are sharded across the 8 NeuronCores.

The slot structure is data-dependent, so the program is built (and cached)
per distinct slot structure; thresholds are baked as immediates.

Shapes are hardcoded for image [6, 2048, 4096] f32 (3 RGB + 3 label chans).
"""

import numpy as np

import concourse.bacc as bacc
import concourse.mybir as mybir
from concourse.tile import TileContext
from concourse import bass_utils

NUM_CH = 6
EQ_CH = 3
H = 2048
W = 4096
NCORES = 8
HSH = H // NCORES          # 256 rows per core
P = 128                    # partitions
NB = 256                   # histogram bins
CW = 2048                  # free-dim chunk (8KB/partition f32)
TWO23 = float(1 << 23)

_CACHED = {}


def _reference_luts(sample_f32):
    """Exact reference LUT math (int64 on host) for the 3 equalize channels.

    Returns luts[3, 256] int64 -- the shifted+clipped LUT, with the
    step==0 identity fallback folded in.
    """
    v = np.floor(sample_f32).astype(np.int64)  # trunc == floor for >=0
    luts = np.zeros((EQ_CH, NB), np.int64)
    for c in range(EQ_CH):
        hist = np.bincount(v[c].ravel(), minlength=NB).astype(np.int64)
        total = int(hist.sum())
        nz = np.nonzero(hist)[0]
        last_nz = int(nz[-1]) if len(nz) else 0
        step = (total - int(hist[last_nz])) // (NB - 1)
        if step == 0:
            luts[c] = np.arange(NB)
            continue
        cum = np.cumsum(hist)
        lut = (cum + step // 2) // step
        lut_shift = np.concatenate([[0], lut[:-1]])
        luts[c] = np.clip(lut_shift, 0, NB - 1)
    return luts


def _slots(luts):
    """Per-channel unit step terms of d[v] = lut[v] - v.

    Returns (slots, consts): slots[c] is an even-length tuple of
    (tau, is_ge) unit steps ((0, True) padding always fires);
    consts[c] = cw = K/2 - #lt - #pad, integral, so that
        lut[v] = (v + cw) + 0.5 * sum_t s_t.
    """
    slots = []
    consts = []
    for c in range(EQ_CH):
        d = luts[c] - np.arange(NB)
        assert d[0] == 0
        jumps = np.diff(d)
        sl = []
        for tau in np.nonzero(jumps)[0]:
            w = int(jumps[tau])
            # +[v >= tau+1] per unit, or +[v < tau+1] (and -1 in const)
            sl.extend([(int(tau) + 1, w > 0)] * abs(w))
        npad = 0
        if len(sl) % 2 == 1:
            sl.append((0, True))
            npad = 1
        nlt = sum(1 for _, ge in sl if not ge)
        slots.append(tuple(sl))
        consts.append(float(len(sl) // 2 - nlt - npad))
    return tuple(slots), tuple(consts)


def _build_kernel(slots, consts):
    """Build the SPMD Bass program (one NEFF, run on all 8 cores)."""
    nc = bacc.Bacc("TRN2", target_bir_lowering=False, debug=False,
                   num_devices=NCORES)
    x = nc.dram_tensor("x", [NUM_CH, HSH, W], mybir.dt.float32,
                       kind="ExternalInput")
    ye = nc.dram_tensor("ye", [EQ_CH, HSH, W], mybir.dt.uint8,
                        kind="ExternalOutput")
    yl = nc.dram_tensor("yl", [NUM_CH - EQ_CH, HSH, W], mybir.dt.float32,
                        kind="ExternalOutput")

    AOT = mybir.AluOpType
    ACF = mybir.ActivationFunctionType
    f32 = mybir.dt.float32
    bf16 = mybir.dt.bfloat16

    # interleaved chunk schedule: channel innermost
    chunks = [(c, h0, w0)
              for h0 in range(0, HSH, P)
              for w0 in range(0, W, CW)
              for c in range(EQ_CH)]
    # label passthrough transfers, interleaved ~1 per 2 chunks
    labels = [(t, h0) for t in range(EQ_CH, NUM_CH)
              for h0 in range(0, HSH, P)]

    with TileContext(nc) as tc:
        with (
            tc.tile_pool(name="io", bufs=4) as io_pool,
            tc.tile_pool(name="wk", bufs=3) as wk_pool,
            tc.tile_pool(name="sg", bufs=4) as sg_pool,
            tc.tile_pool(name="lb", bufs=2) as lb_pool,
            tc.tile_pool(name="cst", bufs=1) as cst_pool,
            tc.tile_pool(name="ps", bufs=2, space="PSUM") as ps_pool,
        ):
            # Sign biases must be [P,1] SBUF operands: memset one per slot
            nslots = max(1, sum(len(s) for s in slots))
            bias_t = cst_pool.tile([P, nslots], f32, tag="bias")
            j = 0
            for c in range(EQ_CH):
                for tau, ge in slots[c]:
                    b = (0.5 - tau - consts[c]) if ge \
                        else (tau - 0.5 + consts[c])
                    nc.vector.memset(bias_t[:, j:j + 1], float(b))
                    j += 1

            # 128x128 identity (bf16) for PE pass-through accumulation
            iot = cst_pool.tile([P, P], mybir.dt.int32, tag="io32")
            nc.gpsimd.iota(iot[:], pattern=[[1, P]], base=0,
                           channel_multiplier=-1)
            ident = cst_pool.tile([P, P], bf16, tag="ident")
            nc.vector.tensor_scalar(ident[:], iot[:], 0.0, None,
                                    AOT.is_equal)
            BANK = 512         # f32 elems per PSUM bank

            def emit_load(i):
                """Stage A: DMA load + ACT r' (pipelined one chunk ahead)."""
                c, h0, w0 = chunks[i]
                xt = io_pool.tile([P, CW], f32, tag="x", name=f"x{i}")
                nc.sync.dma_start(xt[:], x[c, h0:h0 + P, w0:w0 + CW])
                rt = wk_pool.tile([P, CW], f32, tag="r", name=f"r{i}")
                nc.scalar.activation(rt[:], xt[:], ACF.Copy, bias=TWO23)
                return xt, rt

            def emit_label(t, h0):
                lt = lb_pool.tile([P, W], f32, tag="l")
                nc.gpsimd.dma_start(lt[:], x[t, h0:h0 + P, :])
                nc.gpsimd.dma_start(yl[t - EQ_CH, h0:h0 + P, :], lt[:])

            staged = emit_load(0)
            li = 0
            for i, (c, h0, w0) in enumerate(chunks):
                xt, rt = staged
                if i + 1 < len(chunks):
                    staged = emit_load(i + 1)
                if i % 2 == 0 and li < len(labels):
                    emit_label(*labels[li])
                    li += 1

                K = len(slots[c])
                base = sum(len(slots[cc]) for cc in range(c))
                # a0 = [(r' - 2^23) <= x]  (1 - round-up fixup)
                at = wk_pool.tile([P, CW], bf16, tag="a")
                nc.vector.scalar_tensor_tensor(
                    at[:], rt[:], -TWO23, xt[:], AOT.add, AOT.is_le)
                ot = io_pool.tile([P, CW], mybir.dt.uint8, tag="o")
                if K == 0:
                    # identity channel: out = v = (r'-2^23-1) + a0
                    nc.vector.scalar_tensor_tensor(
                        ot[:], rt[:], -(TWO23 + 1.0), at[:],
                        AOT.add, AOT.add)
                else:
                    # v' = (r' - 2^23 - 1 + cw) + a0  (exact, f32)
                    vt = wk_pool.tile([P, CW], f32, tag="v")
                    nc.vector.scalar_tensor_tensor(
                        vt[:], rt[:], consts[c] - (TWO23 + 1.0), at[:],
                        AOT.add, AOT.add)
                    # +-1 indicators on ACT (exact on integer-valued v');
                    # PE sums them into PSUM via identity-weight matmuls
                    pt = ps_pool.tile([P, CW], f32, tag="ps")
                    for k, (tau, ge) in enumerate(slots[c]):
                        st = sg_pool.tile([P, CW], bf16, tag="s")
                        nc.scalar.activation(
                            st[:], vt[:], ACF.Sign,
                            bias=bias_t[:, base + k:base + k + 1],
                            scale=1.0 if ge else -1.0)
                        for j in range(0, CW, BANK):
                            nc.tensor.matmul(
                                pt[:, j:j + BANK], ident[:],
                                st[:, j:j + BANK],
                                start=(k == 0), stop=(k == K - 1))
                    # out = 0.5*U + v'
                    nc.vector.scalar_tensor_tensor(
                        ot[:], pt[:], 0.5, vt[:], AOT.mult, AOT.add)
                nc.gpsimd.dma_start(ye[c, h0:h0 + P, w0:w0 + CW], ot[:])

    nc.finalize()
    return nc


def _prepare(image):
    """Host-side LUT math + program build (cached by slot structure)."""
    luts = _reference_luts(image[:EQ_CH])
    slots, consts = _slots(luts)
    key = (slots, consts)
    if key not in _CACHED:
        _CACHED[key] = _build_kernel(slots, consts)
    return _CACHED[key]


def _in_maps(image):
    return [{"x": np.ascontiguousarray(image[:, i * HSH:(i + 1) * HSH, :])}
            for i in range(NCORES)]


def kernel(image: np.ndarray) -> np.ndarray:
    image = np.ascontiguousarray(image, dtype=np.float32)
    assert image.shape == (NUM_CH, H, W)

    nc = _prepare(image)
    res = bass_utils.run_bass_kernel_spmd(
        nc, _in_maps(image), core_ids=list(range(NCORES)))

    out = np.empty((NUM_CH, H, W), np.float32)
    for i in range(NCORES):
        sl = slice(i * HSH, (i + 1) * HSH)
        out[:EQ_CH, sl] = res.results[i]["ye"].astype(np.float32)
        out[EQ_CH:, sl] = res.results[i]["yl"]
    return out
